# revision 10
# baseline (speedup 1.0000x reference)
"""MoE bias-gather layer (top-2 of 16 experts) as a Bass/Tile kernel on 8
Trainium2 NeuronCores.

Strategy (data parallel over tokens):
  - tokens sharded 8 ways (256 tokens/core); gate weights and expert_biases
    replicated to every core.
  - per core: gate logits via PE matmul (hidden transposed on-device via PE
    transpose), softmax / top-2 / normalized pair weights via DVE+ACT ops,
    then the output  out = W @ expert_biases  as a dense (tokens x 16) @
    (16 x V) bf16 matmul streamed over V, staged through SBUF, DMA'd out.
  - aux-loss partial (per-core sum over tokens of softmax scores) computed
    with a ones-vector matmul; final 16-float reduction + u*log(u) happens
    on host during unsharding.

Layout notes:
  - gate_w is pre-arranged on host to the exact SBUF tile layout
    (128 partitions x [16 H-chunks x 16 experts]) so it loads in one DMA.
  - expert_biases are cast f32->bf16 in-DMA on the SWDGE path so the 2MB
    stream never blocks the HWDGE rings that feed hidden/output traffic.
  - ACT engine runs only copies + Exp (single table set, no DMA issues) --
    sigmoid is computed on DVE from exp values instead.
"""

import sys

sys.path.insert(0, "/opt/trn_rl_repo")

import numpy as np

import concourse.bacc as bacc
import concourse.mybir as mybir
import concourse.tile as tile
from concourse.bass_utils import run_bass_kernel_spmd
from concourse.masks import make_identity

# problem shape (hardcoded per contract)
T, H, V, E = 2048, 2048, 32000, 16
NCORES = 8
TS = T // NCORES      # 256 tokens per core
PT = 128              # tokens per tile (partition dim)
NT = TS // PT         # 2 token tiles per core
HC = H // 128         # 16 contraction chunks for the gate matmul
NCH = 500             # V columns per big matmul (one PSUM bank = 500 fp32)
OUT_BF16 = True       # device writes bf16, host upcasts (halves write stream)
GRP = 16 if OUT_BF16 else 8   # chunks per staging group / out DMA
NGRP = V // (NCH * GRP)  # out-DMA groups per token tile
F32 = mybir.dt.float32
BF16 = mybir.dt.bfloat16
ALU = mybir.AluOpType


def build():
    nc = bacc.Bacc(None, target_bir_lowering=False, debug=False)
    hs = nc.dram_tensor("hidden_states", [TS, H], F32, kind="ExternalInput")
    gw = nc.dram_tensor("gate_w_sb", [128, HC * E], F32, kind="ExternalInput")
    eb = nc.dram_tensor("expert_biases", [E, V], F32, kind="ExternalInput")
    out_dt = BF16 if OUT_BF16 else F32
    out = nc.dram_tensor("out", [TS, V], out_dt, kind="ExternalOutput")
    usage = nc.dram_tensor("usage", [1, E], F32, kind="ExternalOutput")

    with tile.TileContext(nc) as tc:
        with (
            tc.tile_pool(name="const", bufs=1) as constp,
            tc.tile_pool(name="bias", bufs=NGRP) as biasp,
            tc.tile_pool(name="hid", bufs=NT) as hidp,
            tc.tile_pool(name="hidT", bufs=NT) as hidTp,
            tc.tile_pool(name="small", bufs=2) as smallp,
            tc.tile_pool(name="gsmall", bufs=3) as gsp,
            tc.tile_pool(name="stage", bufs=3) as stagep,
            tc.tile_pool(name="psum_g", bufs=2, space="PSUM") as psum_g,
            tc.tile_pool(name="psum_l", bufs=1, space="PSUM") as psum_l,
            tc.tile_pool(name="psum_u", bufs=1, space="PSUM") as psum_u,
            tc.tile_pool(name="psum_b", bufs=2, space="PSUM") as psum_b,
        ):
            identity = constp.tile([128, 128], F32)
            make_identity(nc, identity[:])
            ones = constp.tile([128, 1], F32)
            nc.vector.memset(ones[:], 1.0)

            # gate weights first (tiny, gates all 32 logit matmuls), then
            # hidden tiles (they gate the transposes)
            gwt_sb = constp.tile([128, HC * E], F32)
            nc.sync.dma_start(gwt_sb[:], gw[:])
            hids = []
            for t in range(NT):
                hid = hidp.tile([128, H], F32, tag="hid")
                for k in range(4):
                    w = H // 4
                    nc.sync.dma_start(
                        hid[:, k * w:(k + 1) * w],
                        hs[t * PT:(t + 1) * PT, k * w:(k + 1) * w],
                    )
                hids.append(hid)
            # expert biases: 8 groups of (16, 4000), f32 -> bf16 cast in-DMA
            # on the SWDGE path (keeps the HWDGE rings free)
            bias_tiles = []
            for g in range(NGRP):
                bt = biasp.tile([E, NCH * GRP], BF16, tag="bias")
                nc.gpsimd.dma_start(
                    bt[:], eb[:, g * NCH * GRP:(g + 1) * NCH * GRP]
                )
                bias_tiles.append(bt)

            pu = psum_u.tile([1, E], F32)

            for t in range(NT):
                hid = hids[t]

                # hiddenT chunks via PE transpose; 4 transposes share one
                # PSUM bank, drained by a single wide copy
                hidT = hidTp.tile([128, H], F32, tag="hidT")
                for b in range(HC // 4):
                    tr = psum_g.tile([128, 512], F32, tag="tr")
                    for q in range(4):
                        c = b * 4 + q
                        nc.tensor.transpose(
                            tr[:, q * 128:(q + 1) * 128],
                            hid[:, c * 128:(c + 1) * 128],
                            identity[:],
                        )
                    if b % 2 == 0:
                        nc.vector.tensor_copy(
                            hidT[:, b * 512:(b + 1) * 512], tr[:]
                        )
                    else:
                        nc.scalar.copy(
                            hidT[:, b * 512:(b + 1) * 512], tr[:]
                        )

                # logits (tok x E), fp32 accumulation over HC chunks
                lg = psum_l.tile([128, E], F32, tag="lg")
                for c in range(HC):
                    nc.tensor.matmul(
                        lg[:],
                        hidT[:, c * 128:(c + 1) * 128],
                        gwt_sb[:, c * E:(c + 1) * E],
                        start=(c == 0),
                        stop=(c == HC - 1),
                    )

                # softmax pieces
                negm1 = gsp.tile([128, 1], F32, tag="negm1")
                nc.vector.reduce_max(
                    negm1[:], lg[:], axis=mybir.AxisListType.X, negate=True
                )
                expv = gsp.tile([128, E], F32, tag="expv")
                z = gsp.tile([128, 1], F32, tag="z")
                nc.scalar.activation(
                    expv[:], lg[:], mybir.ActivationFunctionType.Exp,
                    bias=negm1[:], scale=1.0, accum_out=z[:],
                )
                rz = gsp.tile([128, 1], F32, tag="rz")
                nc.vector.reciprocal(rz[:], z[:])
                scoresn = gsp.tile([128, E], F32, tag="scoresn")
                nc.vector.tensor_scalar_mul(scoresn[:], expv[:], rz[:])

                # usage partial: ones^T @ scoresn, accumulated over tiles
                nc.tensor.matmul(
                    pu[:], ones[:], scoresn[:],
                    start=(t == 0), stop=(t == NT - 1),
                )

                # top-2 masks from logits
                m1 = gsp.tile([128, 1], F32, tag="m1")
                nc.vector.tensor_scalar_mul(m1[:], negm1[:], -1.0)
                mask1 = gsp.tile([128, E], F32, tag="mask1")
                nc.vector.tensor_scalar(
                    mask1[:], lg[:], m1[:], None, op0=ALU.is_equal
                )
                msk = gsp.tile([128, E], F32, tag="msk")
                nc.vector.scalar_tensor_tensor(
                    msk[:], mask1[:], -1e30, lg[:],
                    op0=ALU.mult, op1=ALU.add,
                )
                m2 = gsp.tile([128, 1], F32, tag="m2")
                nc.vector.reduce_max(m2[:], msk[:], axis=mybir.AxisListType.X)
                mask2 = gsp.tile([128, E], F32, tag="mask2")
                nc.vector.tensor_scalar(
                    mask2[:], msk[:], m2[:], None, op0=ALU.is_equal
                )

                # pair weights without sigmoid (keeps ACT on one table set):
                # e2 = sum(mask2 * expv) = exp(m2 - m1);  p1 = 1/(1+e2);
                # p2 = e2 * p1
                junk = gsp.tile([128, E], F32, tag="junk")
                e2 = gsp.tile([128, 1], F32, tag="e2")
                nc.vector.scalar_tensor_tensor(
                    junk[:], expv[:], 1.0, mask2[:],
                    op0=ALU.mult, op1=ALU.mult, accum_out=e2[:],
                )
                dn = gsp.tile([128, 1], F32, tag="dn")
                nc.vector.tensor_scalar(
                    dn[:], e2[:], 1.0, None, op0=ALU.add
                )
                p1 = gsp.tile([128, 1], F32, tag="p1")
                nc.vector.reciprocal(p1[:], dn[:])
                p2 = gsp.tile([128, 1], F32, tag="p2")
                nc.vector.tensor_tensor(p2[:], e2[:], p1[:], op=ALU.mult)

                w1 = gsp.tile([128, E], F32, tag="w1")
                nc.vector.tensor_scalar_mul(w1[:], mask1[:], p1[:])
                wfull = gsp.tile([128, E], F32, tag="wfull")
                nc.vector.scalar_tensor_tensor(
                    wfull[:], mask2[:], p2[:], w1[:],
                    op0=ALU.mult, op1=ALU.add,
                )

                # W^T (E x tok) in bf16 for the big matmul
                tw = psum_g.tile([E, 128], F32, tag="tr")
                nc.tensor.transpose(tw[:], wfull[:], identity[:])
                wt_sb = smallp.tile([E, 128], BF16, tag="wt", bufs=NT)
                nc.vector.tensor_copy(wt_sb[:], tw[:])

                # ---- big matmul + writeback for this token tile ----
                # two 500-col matmuls fill a 2-bank PSUM tile; one wide copy
                # (alternating ACT/DVE) drains it into the bf16 staging tile
                for g in range(NGRP):
                    stg = stagep.tile([128, NCH * GRP], out_dt, tag="stg")
                    for pr in range(GRP // 2):
                        # 1024-col psum tile = 2 banks; each 500-col matmul
                        # lands bank-aligned (offsets 0 and 512)
                        pb = psum_b.tile([128, 1024], F32, tag="pb")
                        for h in range(2):
                            j = pr * 2 + h
                            nc.tensor.matmul(
                                pb[:, h * 512:h * 512 + NCH],
                                wt_sb[:],
                                bias_tiles[g][:, j * NCH:(j + 1) * NCH],
                                start=True,
                                stop=True,
                            )
                        src_ap = pb[:].rearrange(
                            "p (a b) -> p a b", a=2, b=512
                        )[:, :, 0:NCH]
                        dst_ap = stg[
                            :, pr * 2 * NCH:(pr + 1) * 2 * NCH
                        ].rearrange("p (a b) -> p a b", a=2, b=NCH)
                        if pr % 2 == 0:
                            nc.scalar.copy(dst_ap, src_ap)
                        else:
                            nc.vector.tensor_copy(dst_ap, src_ap)
                    nc.sync.dma_start(
                        out[t * PT:(t + 1) * PT,
                            g * NCH * GRP:(g + 1) * NCH * GRP],
                        stg[:],
                    )

            # usage -> SBUF -> DRAM
            u_sb = smallp.tile([1, E], F32, tag="usb")
            nc.vector.tensor_copy(u_sb[:], pu[:])
            nc.sync.dma_start(usage[:], u_sb[:])

    nc.compile()
    return nc


_NC_CACHE = None


def _get_nc():
    global _NC_CACHE
    if _NC_CACHE is None:
        _NC_CACHE = build()
    return _NC_CACHE


def _prep_gate_w(gate_w):
    # sb[p, c*16+e] = gate_w[e, c*128+p]
    arr = np.asarray(gate_w, dtype=np.float32).reshape(E, HC, 128)
    return np.ascontiguousarray(arr.transpose(2, 1, 0).reshape(128, HC * E))


def _run(hidden_states, gate_w, expert_biases, trace=False):
    nc = _get_nc()
    hidden_states = np.ascontiguousarray(hidden_states, dtype=np.float32)
    gate_w_sb = _prep_gate_w(gate_w)
    expert_biases = np.ascontiguousarray(expert_biases, dtype=np.float32)
    in_maps = [
        {
            "hidden_states": hidden_states[i * TS:(i + 1) * TS],
            "gate_w_sb": gate_w_sb,
            "expert_biases": expert_biases,
        }
        for i in range(NCORES)
    ]
    res = run_bass_kernel_spmd(
        nc, in_maps, core_ids=list(range(NCORES)), trace=trace
    )
    bias = np.concatenate(
        [np.asarray(r["out"], dtype=np.float32) for r in res.results], axis=0
    )
    usage_sum = np.sum(
        [r["usage"][0] for r in res.results], axis=0, dtype=np.float32
    )
    u = usage_sum / np.float32(T)
    aux = np.float32(np.sum(u * np.log(u)) * E)
    return (bias, aux), res


def kernel(hidden_states, gate_w, expert_biases):
    (bias, aux), _ = _run(hidden_states, gate_w, expert_biases, trace=False)
    return bias, aux


# revision 11
# speedup vs baseline: 1.3413x; 1.3413x over previous
"""MoE bias-gather layer (top-2 of 16 experts) as a Bass/Tile kernel on 8
Trainium2 NeuronCores.

Strategy (data parallel over tokens):
  - tokens sharded 8 ways (256 tokens/core); gate weights and expert_biases
    replicated to every core.
  - per core: gate logits via PE matmul (hidden transposed on-device via PE
    transpose), softmax / top-2 / normalized pair weights via DVE+ACT ops,
    then the output  out = W @ expert_biases  as a dense (tokens x 16) @
    (16 x V) bf16 matmul streamed over V, staged through SBUF, DMA'd out.
  - aux-loss partial (per-core sum over tokens of softmax scores) computed
    with a ones-vector matmul; final 16-float reduction + u*log(u) happens
    on host during unsharding.

Layout notes:
  - gate_w is pre-arranged on host to the exact SBUF tile layout
    (128 partitions x [16 H-chunks x 16 experts]) so it loads in one DMA.
  - expert_biases are cast f32->bf16 in-DMA on the SWDGE path so the 2MB
    stream never blocks the HWDGE rings that feed hidden/output traffic.
  - ACT engine runs only copies + Exp (single table set, no DMA issues) --
    sigmoid is computed on DVE from exp values instead.
"""

import sys

sys.path.insert(0, "/opt/trn_rl_repo")

import numpy as np

import concourse.bacc as bacc
import concourse.mybir as mybir
import concourse.tile as tile
from concourse.bass_utils import run_bass_kernel_spmd
from concourse.masks import make_identity

# problem shape (hardcoded per contract)
T, H, V, E = 2048, 2048, 32000, 16
NCORES = 8
TS = T // NCORES      # 256 tokens per core
PT = 128              # tokens per tile (partition dim)
NT = TS // PT         # 2 token tiles per core
HC = H // 128         # 16 contraction chunks for the gate matmul
NCH = 500             # V columns per big matmul (one PSUM bank = 500 fp32)
OUT_BF16 = True       # device writes bf16, host upcasts (halves write stream)
GRP = 16 if OUT_BF16 else 8   # chunks per staging group / out DMA
NGRP = V // (NCH * GRP)  # out-DMA groups per token tile
F32 = mybir.dt.float32
BF16 = mybir.dt.bfloat16
ALU = mybir.AluOpType


def build():
    nc = bacc.Bacc(None, target_bir_lowering=False, debug=False)
    hs = nc.dram_tensor("hidden_states", [TS, H], F32, kind="ExternalInput")
    gw = nc.dram_tensor("gate_w_sb", [128, HC * E], F32, kind="ExternalInput")
    eb = nc.dram_tensor("expert_biases", [E, V], F32, kind="ExternalInput")
    out_dt = BF16 if OUT_BF16 else F32
    out = nc.dram_tensor("out", [TS, V], out_dt, kind="ExternalOutput")
    usage = nc.dram_tensor("usage", [1, E], F32, kind="ExternalOutput")

    with tile.TileContext(nc) as tc:
        with (
            tc.tile_pool(name="const", bufs=1) as constp,
            tc.tile_pool(name="bias", bufs=NGRP) as biasp,
            tc.tile_pool(name="hid", bufs=NT) as hidp,
            tc.tile_pool(name="hidT", bufs=NT) as hidTp,
            tc.tile_pool(name="small", bufs=2) as smallp,
            tc.tile_pool(name="gsmall", bufs=3) as gsp,
            tc.tile_pool(name="stage", bufs=3) as stagep,
            tc.tile_pool(name="psum_g", bufs=2, space="PSUM") as psum_g,
            tc.tile_pool(name="psum_l", bufs=1, space="PSUM") as psum_l,
            tc.tile_pool(name="psum_u", bufs=1, space="PSUM") as psum_u,
            tc.tile_pool(name="psum_b", bufs=4, space="PSUM") as psum_b,
        ):
            identity = constp.tile([128, 128], F32)
            make_identity(nc, identity[:])
            ones = constp.tile([128, 1], F32)
            nc.vector.memset(ones[:], 1.0)

            # gate weights first (tiny, gates all 32 logit matmuls), then
            # hidden tiles (they gate the transposes)
            gwt_sb = constp.tile([128, HC * E], F32)
            nc.sync.dma_start(gwt_sb[:], gw[:])
            hids = []
            for t in range(NT):
                hid = hidp.tile([128, H], F32, tag="hid")
                for k in range(4):
                    w = H // 4
                    nc.sync.dma_start(
                        hid[:, k * w:(k + 1) * w],
                        hs[t * PT:(t + 1) * PT, k * w:(k + 1) * w],
                    )
                hids.append(hid)
            # expert biases: 8 groups of (16, 4000), f32 -> bf16 cast in-DMA
            # on the SWDGE path (keeps the HWDGE rings free)
            bias_tiles = []
            for g in range(NGRP):
                bt = biasp.tile([E, NCH * GRP], BF16, tag="bias")
                nc.gpsimd.dma_start(
                    bt[:], eb[:, g * NCH * GRP:(g + 1) * NCH * GRP]
                )
                bias_tiles.append(bt)

            pu = psum_u.tile([1, E], F32)

            wt_sbs = [None, None]

            def gate_closures(t):
                """Emit-closures for token tile t's gate phase, granular
                enough to interleave into the other tile's big-mm stream."""
                hid = hids[t]
                hidT = hidTp.tile([128, H], F32, tag="hidT")
                lg = psum_l.tile([128, E], F32, tag="lg")
                ops = []

                def transpose_batch(b):
                    def _f():
                        tr = psum_g.tile([128, 512], F32, tag="tr")
                        for q in range(4):
                            c = b * 4 + q
                            nc.tensor.transpose(
                                tr[:, q * 128:(q + 1) * 128],
                                hid[:, c * 128:(c + 1) * 128],
                                identity[:],
                            )
                        if b % 2 == 0:
                            nc.vector.tensor_copy(
                                hidT[:, b * 512:(b + 1) * 512], tr[:]
                            )
                        else:
                            nc.scalar.copy(
                                hidT[:, b * 512:(b + 1) * 512], tr[:]
                            )
                    return _f

                for b in range(HC // 4):
                    ops.append(transpose_batch(b))

                def gate_mms(c0):
                    def _f():
                        for c in range(c0, c0 + 4):
                            nc.tensor.matmul(
                                lg[:],
                                hidT[:, c * 128:(c + 1) * 128],
                                gwt_sb[:, c * E:(c + 1) * E],
                                start=(c == 0),
                                stop=(c == HC - 1),
                            )
                    return _f

                for c0 in range(0, HC, 4):
                    ops.append(gate_mms(c0))

                def softmax_part():
                    negm1 = gsp.tile([128, 1], F32, tag="negm1")
                    nc.vector.reduce_max(
                        negm1[:], lg[:], axis=mybir.AxisListType.X, negate=True
                    )
                    expv = gsp.tile([128, E], F32, tag="expv")
                    z = gsp.tile([128, 1], F32, tag="z")
                    nc.scalar.activation(
                        expv[:], lg[:], mybir.ActivationFunctionType.Exp,
                        bias=negm1[:], scale=1.0, accum_out=z[:],
                    )
                    rz = gsp.tile([128, 1], F32, tag="rz")
                    nc.vector.reciprocal(rz[:], z[:])
                    scoresn = gsp.tile([128, E], F32, tag="scoresn")
                    nc.vector.tensor_scalar_mul(scoresn[:], expv[:], rz[:])
                    nc.tensor.matmul(
                        pu[:], ones[:], scoresn[:],
                        start=(t == 0), stop=(t == NT - 1),
                    )
                    gate_closures.saved[t] = (negm1, expv)

                ops.append(softmax_part)

                def top2_part():
                    negm1, expv = gate_closures.saved[t]
                    m1 = gsp.tile([128, 1], F32, tag="m1")
                    nc.vector.tensor_scalar_mul(m1[:], negm1[:], -1.0)
                    mask1 = gsp.tile([128, E], F32, tag="mask1")
                    nc.vector.tensor_scalar(
                        mask1[:], lg[:], m1[:], None, op0=ALU.is_equal
                    )
                    msk = gsp.tile([128, E], F32, tag="msk")
                    nc.vector.scalar_tensor_tensor(
                        msk[:], mask1[:], -1e30, lg[:],
                        op0=ALU.mult, op1=ALU.add,
                    )
                    m2 = gsp.tile([128, 1], F32, tag="m2")
                    nc.vector.reduce_max(
                        m2[:], msk[:], axis=mybir.AxisListType.X
                    )
                    mask2 = gsp.tile([128, E], F32, tag="mask2")
                    nc.vector.tensor_scalar(
                        mask2[:], msk[:], m2[:], None, op0=ALU.is_equal
                    )
                    junk = gsp.tile([128, E], F32, tag="junk")
                    e2 = gsp.tile([128, 1], F32, tag="e2")
                    nc.vector.scalar_tensor_tensor(
                        junk[:], expv[:], 1.0, mask2[:],
                        op0=ALU.mult, op1=ALU.mult, accum_out=e2[:],
                    )
                    dn = gsp.tile([128, 1], F32, tag="dn")
                    nc.vector.tensor_scalar(
                        dn[:], e2[:], 1.0, None, op0=ALU.add
                    )
                    p1 = gsp.tile([128, 1], F32, tag="p1")
                    nc.vector.reciprocal(p1[:], dn[:])
                    p2 = gsp.tile([128, 1], F32, tag="p2")
                    nc.vector.tensor_tensor(p2[:], e2[:], p1[:], op=ALU.mult)
                    w1 = gsp.tile([128, E], F32, tag="w1")
                    nc.vector.tensor_scalar_mul(w1[:], mask1[:], p1[:])
                    wfull = gsp.tile([128, E], F32, tag="wfull")
                    nc.vector.scalar_tensor_tensor(
                        wfull[:], mask2[:], p2[:], w1[:],
                        op0=ALU.mult, op1=ALU.add,
                    )
                    tw = psum_g.tile([E, 128], F32, tag="tr")
                    nc.tensor.transpose(tw[:], wfull[:], identity[:])
                    wt_sb = smallp.tile([E, 128], BF16, tag="wt", bufs=NT)
                    nc.vector.tensor_copy(wt_sb[:], tw[:])
                    wt_sbs[t] = wt_sb

                ops.append(top2_part)
                return ops

            gate_closures.saved = {}

            def emit_big(t, interleave=None):
                """Big matmul + writeback for tile t; pops one closure from
                `interleave` after every other chunk to hide the next tile's
                gate phase inside this tile's copy/DMA-bound stream."""
                wt_sb = wt_sbs[t]
                k = 0
                for g in range(NGRP):
                    stg = stagep.tile([128, NCH * GRP], out_dt, tag="stg")
                    for j in range(GRP):
                        pb = psum_b.tile([128, NCH], F32, tag="pb")
                        nc.tensor.matmul(
                            pb[:],
                            wt_sb[:],
                            bias_tiles[g][:, j * NCH:(j + 1) * NCH],
                            start=True,
                            stop=True,
                        )
                        if j % 2 == 0:
                            nc.scalar.copy(
                                stg[:, j * NCH:(j + 1) * NCH], pb[:]
                            )
                        else:
                            nc.vector.tensor_copy(
                                stg[:, j * NCH:(j + 1) * NCH], pb[:]
                            )
                        k += 1
                        if interleave and k % 2 == 0:
                            interleave.pop(0)()
                    nc.sync.dma_start(
                        out[t * PT:(t + 1) * PT,
                            g * NCH * GRP:(g + 1) * NCH * GRP],
                        stg[:],
                    )
                if interleave:
                    for op in interleave:
                        op()
                    interleave.clear()

            for op in gate_closures(0):
                op()
            emit_big(0, interleave=gate_closures(1))
            emit_big(1)

            # usage -> SBUF -> DRAM
            u_sb = smallp.tile([1, E], F32, tag="usb")
            nc.vector.tensor_copy(u_sb[:], pu[:])
            nc.sync.dma_start(usage[:], u_sb[:])

    nc.compile()
    return nc


_NC_CACHE = None


def _get_nc():
    global _NC_CACHE
    if _NC_CACHE is None:
        _NC_CACHE = build()
    return _NC_CACHE


def _prep_gate_w(gate_w):
    # sb[p, c*16+e] = gate_w[e, c*128+p]
    arr = np.asarray(gate_w, dtype=np.float32).reshape(E, HC, 128)
    return np.ascontiguousarray(arr.transpose(2, 1, 0).reshape(128, HC * E))


def _run(hidden_states, gate_w, expert_biases, trace=False):
    nc = _get_nc()
    hidden_states = np.ascontiguousarray(hidden_states, dtype=np.float32)
    gate_w_sb = _prep_gate_w(gate_w)
    expert_biases = np.ascontiguousarray(expert_biases, dtype=np.float32)
    in_maps = [
        {
            "hidden_states": hidden_states[i * TS:(i + 1) * TS],
            "gate_w_sb": gate_w_sb,
            "expert_biases": expert_biases,
        }
        for i in range(NCORES)
    ]
    res = run_bass_kernel_spmd(
        nc, in_maps, core_ids=list(range(NCORES)), trace=trace
    )
    bias = np.concatenate(
        [np.asarray(r["out"], dtype=np.float32) for r in res.results], axis=0
    )
    usage_sum = np.sum(
        [r["usage"][0] for r in res.results], axis=0, dtype=np.float32
    )
    u = usage_sum / np.float32(T)
    aux = np.float32(np.sum(u * np.log(u)) * E)
    return (bias, aux), res


def kernel(hidden_states, gate_w, expert_biases):
    (bias, aux), _ = _run(hidden_states, gate_w, expert_biases, trace=False)
    return bias, aux


# revision 12
# speedup vs baseline: 1.3849x; 1.0325x over previous
"""MoE bias-gather layer (top-2 of 16 experts) as a Bass/Tile kernel on 8
Trainium2 NeuronCores.

Strategy (data parallel over tokens):
  - tokens sharded 8 ways (256 tokens/core); gate weights and expert_biases
    replicated to every core.
  - per core: gate logits via PE matmul (hidden transposed on-device via PE
    transpose), softmax / top-2 / normalized pair weights via DVE+ACT ops,
    then the output  out = W @ expert_biases  as a dense (tokens x 16) @
    (16 x V) bf16 matmul streamed over V, staged through SBUF, DMA'd out.
  - aux-loss partial (per-core sum over tokens of softmax scores) computed
    with a ones-vector matmul; final 16-float reduction + u*log(u) happens
    on host during unsharding.

Layout notes:
  - gate_w is pre-arranged on host to the exact SBUF tile layout
    (128 partitions x [16 H-chunks x 16 experts]) so it loads in one DMA.
  - expert_biases are cast f32->bf16 in-DMA on the SWDGE path so the 2MB
    stream never blocks the HWDGE rings that feed hidden/output traffic.
  - ACT engine runs only copies + Exp (single table set, no DMA issues) --
    sigmoid is computed on DVE from exp values instead.
"""

import sys

sys.path.insert(0, "/opt/trn_rl_repo")

import numpy as np

import concourse.bacc as bacc
import concourse.mybir as mybir
import concourse.tile as tile
from concourse.bass_utils import run_bass_kernel_spmd
from concourse.masks import make_identity

# problem shape (hardcoded per contract)
T, H, V, E = 2048, 2048, 32000, 16
NCORES = 8
TS = T // NCORES      # 256 tokens per core
PT = 128              # tokens per tile (partition dim)
NT = TS // PT         # 2 token tiles per core
HC = H // 128         # 16 contraction chunks for the gate matmul
NCH = 500             # V columns per big matmul (one PSUM bank = 500 fp32)
OUT_BF16 = True       # device writes bf16, host upcasts (halves write stream)
GRP = 16 if OUT_BF16 else 8   # chunks per staging group / out DMA
NGRP = V // (NCH * GRP)  # out-DMA groups per token tile
F32 = mybir.dt.float32
BF16 = mybir.dt.bfloat16
ALU = mybir.AluOpType


def build():
    nc = bacc.Bacc(None, target_bir_lowering=False, debug=False)
    hs = nc.dram_tensor("hidden_states", [TS, H], F32, kind="ExternalInput")
    gw = nc.dram_tensor("gate_w_sb", [128, HC * E], F32, kind="ExternalInput")
    eb = nc.dram_tensor("expert_biases", [E, V], F32, kind="ExternalInput")
    out_dt = BF16 if OUT_BF16 else F32
    out = nc.dram_tensor("out", [TS, V], out_dt, kind="ExternalOutput")
    usage = nc.dram_tensor("usage", [1, E], F32, kind="ExternalOutput")

    with tile.TileContext(nc) as tc:
        with (
            tc.tile_pool(name="const", bufs=1) as constp,
            tc.tile_pool(name="bias", bufs=NGRP) as biasp,
            tc.tile_pool(name="hid", bufs=NT) as hidp,
            tc.tile_pool(name="hidT", bufs=NT) as hidTp,
            tc.tile_pool(name="small", bufs=2) as smallp,
            tc.tile_pool(name="gsmall", bufs=3) as gsp,
            tc.tile_pool(name="stage", bufs=3) as stagep,
            tc.tile_pool(name="psum_g", bufs=2, space="PSUM") as psum_g,
            tc.tile_pool(name="psum_l", bufs=1, space="PSUM") as psum_l,
            tc.tile_pool(name="psum_u", bufs=1, space="PSUM") as psum_u,
            tc.tile_pool(name="psum_b", bufs=4, space="PSUM") as psum_b,
        ):
            identity = constp.tile([128, 128], F32)
            make_identity(nc, identity[:])
            ones = constp.tile([128, 1], F32)
            nc.vector.memset(ones[:], 1.0)

            # gate weights first (tiny, gates all 32 logit matmuls), then
            # hidden tiles (they gate the transposes)
            gwt_sb = constp.tile([128, HC * E], F32)
            nc.sync.dma_start(gwt_sb[:], gw[:])
            hids = []
            for t in range(NT):
                hid = hidp.tile([128, H], F32, tag="hid")
                nsub = 2 if t == 0 else 1
                w = H // nsub
                for k in range(nsub):
                    nc.sync.dma_start(
                        hid[:, k * w:(k + 1) * w],
                        hs[t * PT:(t + 1) * PT, k * w:(k + 1) * w],
                    )
                hids.append(hid)
            # expert biases: 8 groups of (16, 4000), f32 -> bf16 cast in-DMA
            # on the SWDGE path (keeps the HWDGE rings free)
            bias_tiles = []
            for g in range(NGRP):
                bt = biasp.tile([E, NCH * GRP], BF16, tag="bias")
                nc.gpsimd.dma_start(
                    bt[:], eb[:, g * NCH * GRP:(g + 1) * NCH * GRP]
                )
                bias_tiles.append(bt)

            pu = psum_u.tile([1, E], F32)

            wt_sbs = [None, None]

            def gate_closures(t):
                """Emit-closures for token tile t's gate phase, granular
                enough to interleave into the other tile's big-mm stream."""
                hid = hids[t]
                hidT = hidTp.tile([128, H], F32, tag="hidT")
                lg = psum_l.tile([128, E], F32, tag="lg")
                ops = []

                def transpose_batch(b):
                    def _f():
                        tr = psum_g.tile([128, 512], F32, tag="tr")
                        for q in range(4):
                            c = b * 4 + q
                            nc.tensor.transpose(
                                tr[:, q * 128:(q + 1) * 128],
                                hid[:, c * 128:(c + 1) * 128],
                                identity[:],
                            )
                        if b % 2 == 0:
                            nc.vector.tensor_copy(
                                hidT[:, b * 512:(b + 1) * 512], tr[:]
                            )
                        else:
                            nc.scalar.copy(
                                hidT[:, b * 512:(b + 1) * 512], tr[:]
                            )
                    return _f

                for b in range(HC // 4):
                    ops.append(transpose_batch(b))

                def gate_mms(c0):
                    def _f():
                        for c in range(c0, c0 + 4):
                            nc.tensor.matmul(
                                lg[:],
                                hidT[:, c * 128:(c + 1) * 128],
                                gwt_sb[:, c * E:(c + 1) * E],
                                start=(c == 0),
                                stop=(c == HC - 1),
                            )
                    return _f

                for c0 in range(0, HC, 4):
                    ops.append(gate_mms(c0))

                def softmax_part():
                    negm1 = gsp.tile([128, 1], F32, tag="negm1")
                    nc.vector.reduce_max(
                        negm1[:], lg[:], axis=mybir.AxisListType.X, negate=True
                    )
                    expv = gsp.tile([128, E], F32, tag="expv")
                    z = gsp.tile([128, 1], F32, tag="z")
                    nc.scalar.activation(
                        expv[:], lg[:], mybir.ActivationFunctionType.Exp,
                        bias=negm1[:], scale=1.0, accum_out=z[:],
                    )
                    rz = gsp.tile([128, 1], F32, tag="rz")
                    nc.vector.reciprocal(rz[:], z[:])
                    scoresn = gsp.tile([128, E], F32, tag="scoresn")
                    nc.vector.tensor_scalar_mul(scoresn[:], expv[:], rz[:])
                    nc.tensor.matmul(
                        pu[:], ones[:], scoresn[:],
                        start=(t == 0), stop=(t == NT - 1),
                    )
                    gate_closures.saved[t] = (negm1, expv)

                ops.append(softmax_part)

                def top2_part():
                    negm1, expv = gate_closures.saved[t]
                    m1 = gsp.tile([128, 1], F32, tag="m1")
                    nc.vector.tensor_scalar_mul(m1[:], negm1[:], -1.0)
                    mask1 = gsp.tile([128, E], F32, tag="mask1")
                    nc.vector.tensor_scalar(
                        mask1[:], lg[:], m1[:], None, op0=ALU.is_equal
                    )
                    msk = gsp.tile([128, E], F32, tag="msk")
                    nc.vector.scalar_tensor_tensor(
                        msk[:], mask1[:], -1e30, lg[:],
                        op0=ALU.mult, op1=ALU.add,
                    )
                    m2 = gsp.tile([128, 1], F32, tag="m2")
                    nc.vector.reduce_max(
                        m2[:], msk[:], axis=mybir.AxisListType.X
                    )
                    mask2 = gsp.tile([128, E], F32, tag="mask2")
                    nc.vector.tensor_scalar(
                        mask2[:], msk[:], m2[:], None, op0=ALU.is_equal
                    )
                    junk = gsp.tile([128, E], F32, tag="junk")
                    e2 = gsp.tile([128, 1], F32, tag="e2")
                    nc.vector.scalar_tensor_tensor(
                        junk[:], expv[:], 1.0, mask2[:],
                        op0=ALU.mult, op1=ALU.mult, accum_out=e2[:],
                    )
                    dn = gsp.tile([128, 1], F32, tag="dn")
                    nc.vector.tensor_scalar(
                        dn[:], e2[:], 1.0, None, op0=ALU.add
                    )
                    p1 = gsp.tile([128, 1], F32, tag="p1")
                    nc.vector.reciprocal(p1[:], dn[:])
                    p2 = gsp.tile([128, 1], F32, tag="p2")
                    nc.vector.tensor_tensor(p2[:], e2[:], p1[:], op=ALU.mult)
                    w1 = gsp.tile([128, E], F32, tag="w1")
                    nc.vector.tensor_scalar_mul(w1[:], mask1[:], p1[:])
                    wfull = gsp.tile([128, E], F32, tag="wfull")
                    nc.vector.scalar_tensor_tensor(
                        wfull[:], mask2[:], p2[:], w1[:],
                        op0=ALU.mult, op1=ALU.add,
                    )
                    tw = psum_g.tile([E, 128], F32, tag="tr")
                    nc.tensor.transpose(tw[:], wfull[:], identity[:])
                    wt_sb = smallp.tile([E, 128], BF16, tag="wt", bufs=NT)
                    nc.vector.tensor_copy(wt_sb[:], tw[:])
                    wt_sbs[t] = wt_sb

                ops.append(top2_part)
                return ops

            gate_closures.saved = {}

            def emit_big(t, interleave=None):
                """Big matmul + writeback for tile t; pops one closure from
                `interleave` after every other chunk to hide the next tile's
                gate phase inside this tile's copy/DMA-bound stream."""
                wt_sb = wt_sbs[t]
                k = 0
                for g in range(NGRP):
                    stg = stagep.tile([128, NCH * GRP], out_dt, tag="stg")
                    for j in range(GRP):
                        pb = psum_b.tile([128, NCH], F32, tag="pb")
                        nc.tensor.matmul(
                            pb[:],
                            wt_sb[:],
                            bias_tiles[g][:, j * NCH:(j + 1) * NCH],
                            start=True,
                            stop=True,
                        )
                        if j % 2 == 0:
                            nc.scalar.copy(
                                stg[:, j * NCH:(j + 1) * NCH], pb[:]
                            )
                        else:
                            nc.vector.tensor_copy(
                                stg[:, j * NCH:(j + 1) * NCH], pb[:]
                            )
                        k += 1
                        if interleave and k % 4 == 0:
                            interleave.pop(0)()
                    c0 = g * NCH * GRP
                    if g == NGRP - 1:
                        hw = NCH * GRP // 2
                        nc.sync.dma_start(
                            out[t * PT:(t + 1) * PT, c0:c0 + hw],
                            stg[:, 0:hw],
                        )
                        nc.sync.dma_start(
                            out[t * PT:(t + 1) * PT, c0 + hw:c0 + 2 * hw],
                            stg[:, hw:2 * hw],
                        )
                    else:
                        nc.sync.dma_start(
                            out[t * PT:(t + 1) * PT, c0:c0 + NCH * GRP],
                            stg[:],
                        )
                if interleave:
                    for op in interleave:
                        op()
                    interleave.clear()

            for op in gate_closures(0):
                op()
            emit_big(0, interleave=gate_closures(1))
            emit_big(1)

            # usage -> SBUF -> DRAM
            u_sb = smallp.tile([1, E], F32, tag="usb")
            nc.vector.tensor_copy(u_sb[:], pu[:])
            nc.sync.dma_start(usage[:], u_sb[:])

    nc.compile()
    return nc


_NC_CACHE = None


def _get_nc():
    global _NC_CACHE
    if _NC_CACHE is None:
        _NC_CACHE = build()
    return _NC_CACHE


def _prep_gate_w(gate_w):
    # sb[p, c*16+e] = gate_w[e, c*128+p]
    arr = np.asarray(gate_w, dtype=np.float32).reshape(E, HC, 128)
    return np.ascontiguousarray(arr.transpose(2, 1, 0).reshape(128, HC * E))


def _run(hidden_states, gate_w, expert_biases, trace=False):
    nc = _get_nc()
    hidden_states = np.ascontiguousarray(hidden_states, dtype=np.float32)
    gate_w_sb = _prep_gate_w(gate_w)
    expert_biases = np.ascontiguousarray(expert_biases, dtype=np.float32)
    in_maps = [
        {
            "hidden_states": hidden_states[i * TS:(i + 1) * TS],
            "gate_w_sb": gate_w_sb,
            "expert_biases": expert_biases,
        }
        for i in range(NCORES)
    ]
    res = run_bass_kernel_spmd(
        nc, in_maps, core_ids=list(range(NCORES)), trace=trace
    )
    bias = np.concatenate(
        [np.asarray(r["out"], dtype=np.float32) for r in res.results], axis=0
    )
    usage_sum = np.sum(
        [r["usage"][0] for r in res.results], axis=0, dtype=np.float32
    )
    u = usage_sum / np.float32(T)
    aux = np.float32(np.sum(u * np.log(u)) * E)
    return (bias, aux), res


def kernel(hidden_states, gate_w, expert_biases):
    (bias, aux), _ = _run(hidden_states, gate_w, expert_biases, trace=False)
    return bias, aux
